# revision 4
# baseline (speedup 1.0000x reference)
"""BiLinearInteraction Trainium2 kernel (8 NeuronCores, data-parallel over batch).

Reference computation (per pair p=(i,j) of F=26 fields, P=325 pairs):
    out[b, p*64:(p+1)*64] = (x[i, b, :] @ W[p]) * x[j, b, :]
Full shapes: x [26, 4096, 64] f32, W [325, 64, 64] f32 -> out [4096, 20800] f32.

Strategy (v2)
- Shard batch 4096 -> 8 x 512 (4 tiles of 128 rows/core), replicate W.
- HBM traffic minimized to 27.4 MB/core: out 21.3MB bf16 write + reads 6.06MB
  (xn 1.7 + xt 1.7 + w 2.66, all single-copy bf16). Even fields' matmul
  operands (lhsT xt, rhs w) are packed to SBUF partitions 0-63 and odd fields'
  to 64-127, so the PE 2-row-group concurrency (tile_position row tiling)
  needs no duplicated HBM copies; consecutive fields' matmul pieces are
  emitted interleaved so the two 64-row PE groups stream concurrently.
- Elementwise (the (..)*xj mul, 83.2K col-cycles/core) is split across ACT
  and DVE to balance ~68us each: "drained" fields go PSUM -f32->bf16-> stage
  (ACT copy) then an in-place all-bf16 SBUF DVE mul (2x packed mode);
  "direct" fields do a single 1x DVE mul straight from PSUM f32. Assignment
  is a greedy makespan balance over field sizes.
- Output staged per (tile, chunk-of-fields) and written as 7 contiguous
  bf16 DMAs/tile on the SP HWDGE ring (160KB-1.1MB each); first chunk is a
  single field so the write stream starts ~10us in; input loads ride SWDGE
  (gpsimd) so they never queue behind output writes.
"""

import sys

sys.path.insert(0, "/opt/trn_rl_repo")

import ml_dtypes
import numpy as np

import concourse.bass as bass
import concourse.mybir as mybir
from concourse import bacc
from concourse.tile import TileContext

from itertools import combinations

F, D, B = 26, 64, 4096
NCORES = 8
BC = B // NCORES          # 512 batch rows per core
NT = BC // 128            # 4 batch tiles of 128 rows
NF = F - 1                # 25 left fields
PAIRS = list(combinations(range(F), 2))
N_PAIRS = [F - 1 - i for i in range(NF)]            # pairs with left field i
P_START = [sum(N_PAIRS[:i]) for i in range(NF)]     # first pair index of field i
P = sum(N_PAIRS)          # 325
OUT_COLS = P * D          # 20800

EVEN = [i for i in range(NF) if i % 2 == 0]         # -> partitions 0-63
ODD = [i for i in range(NF) if i % 2 == 1]          # -> partitions 64-127
LO_COLS = sum(N_PAIRS[i] for i in EVEN) * D         # 10816
HI_COLS = sum(N_PAIRS[i] for i in ODD) * D          # 9984
# column offset of field i inside w_lo / w_hi (pair-grouped, parity-packed)
WOFF = {}
off_lo = off_hi = 0
for i in range(NF):
    if i % 2 == 0:
        WOFF[i] = off_lo
        off_lo += N_PAIRS[i] * D
    else:
        WOFF[i] = off_hi
        off_hi += N_PAIRS[i] * D
XT_LO_FIELDS = len(EVEN)   # 13
XT_HI_FIELDS = len(ODD)    # 12

# Output chunks: contiguous field ranges. First/last chunks small so the
# SP write stream starts early and drains quickly at the tail.
CHUNKS = [(0, 1), (1, 3), (3, 6), (6, 10), (10, 15), (15, 21), (21, 25)]

# Greedy ACT/DVE makespan balance: drained fields cost ACT 0.833ns/col
# (+250ns) and DVE 0.52ns/col (2x bf16, +130ns); direct fields cost DVE
# 1.04ns/col from PSUM (+190ns).
DRAINED = {}
_act = _dve = 0.0
for _i in range(NF):
    _u = D * N_PAIRS[_i]
    if _act + 0.833 * _u + 250 <= _dve + 0.52 * _u + 60:
        DRAINED[_i] = True
        _act += 0.833 * _u + 250
        _dve += 0.52 * _u + 130
    else:
        DRAINED[_i] = False
        _dve += 1.04 * _u + 190

F32 = mybir.dt.float32
BF16 = mybir.dt.bfloat16


def build_bass() -> bass.Bass:
    # Bacc (not Bass): its compile() splits multi-sem waits into event
    # semaphores - TRN2 engine instructions take at most one inline wait.
    nc = bacc.Bacc()
    xn = nc.declare_dram_parameter("xn", [BC, F * D], BF16, isOutput=False)
    xt_lo = nc.declare_dram_parameter(
        "xt_lo", [D, NT * XT_LO_FIELDS * 128], BF16, isOutput=False)
    xt_hi = nc.declare_dram_parameter(
        "xt_hi", [D, NT * XT_HI_FIELDS * 128], BF16, isOutput=False)
    w_lo = nc.declare_dram_parameter("w_lo", [D, LO_COLS], BF16, isOutput=False)
    w_hi = nc.declare_dram_parameter("w_hi", [D, HI_COLS], BF16, isOutput=False)
    out = nc.declare_dram_parameter("out", [BC, OUT_COLS], BF16, isOutput=True)

    # Per-chunk w tile column extents (lo on partitions 0-63, hi on 64-127).
    chunk_lo = []  # (first even field, lo cols) per chunk
    chunk_hi = []
    for f0, f1 in CHUNKS:
        ev = [i for i in range(f0, f1) if i % 2 == 0]
        od = [i for i in range(f0, f1) if i % 2 == 1]
        chunk_lo.append((ev[0] if ev else None,
                         sum(N_PAIRS[i] for i in ev) * D))
        chunk_hi.append((od[0] if od else None,
                         sum(N_PAIRS[i] for i in od) * D))

    with TileContext(nc) as tc:
        with (
            tc.tile_pool(name="consts", bufs=1) as consts,
            tc.tile_pool(name="xn_pool", bufs=2) as xn_pool,
            tc.tile_pool(name="xt_pool", bufs=2) as xt_pool,
            tc.tile_pool(name="stage", bufs=2) as stage_pool,
            tc.tile_pool(name="psum", bufs=2, space="PSUM") as psum_pool,
        ):
            w_sb = [consts.tile([2 * D, max(lc, hc)], BF16,
                                tag=f"w{ci}", name=f"w{ci}")
                    for ci, ((_, lc), (_, hc)) in enumerate(zip(chunk_lo, chunk_hi))]
            xn_sb = [xn_pool.tile([128, F * D], BF16, tag="xn", name=f"xn{t}")
                     for t in range(NT)]
            xt_sb = [xt_pool.tile([2 * D, XT_LO_FIELDS * 128], BF16,
                                  tag="xt", name=f"xt{t}")
                     for t in range(NT)]

            # All input loads on SWDGE (gpsimd), issued in just-in-time
            # order: first chunk's operands first, later tiles last.
            def load_w_chunk(ci):
                (elo, lc), (ohi, hc) = chunk_lo[ci], chunk_hi[ci]
                if lc:
                    nc.gpsimd.dma_start(
                        out=w_sb[ci][0:D, 0:lc],
                        in_=w_lo[:, WOFF[elo]:WOFF[elo] + lc])
                if hc:
                    nc.gpsimd.dma_start(
                        out=w_sb[ci][D:2 * D, 0:hc],
                        in_=w_hi[:, WOFF[ohi]:WOFF[ohi] + hc])

            def load_tile(t):
                nc.gpsimd.dma_start(
                    out=xn_sb[t][:], in_=xn[t * 128:(t + 1) * 128, :])
                s = t * XT_LO_FIELDS * 128
                nc.gpsimd.dma_start(
                    out=xt_sb[t][0:D, 0:XT_LO_FIELDS * 128],
                    in_=xt_lo[:, s:s + XT_LO_FIELDS * 128])
                s = t * XT_HI_FIELDS * 128
                nc.gpsimd.dma_start(
                    out=xt_sb[t][D:2 * D, 0:XT_HI_FIELDS * 128],
                    in_=xt_hi[:, s:s + XT_HI_FIELDS * 128])

            load_w_chunk(0)
            load_tile(0)
            for ci in range(1, len(CHUNKS)):
                load_w_chunk(ci)
            for t in range(1, NT):
                load_tile(t)

            field_chunk = {}
            for ci, (f0, f1) in enumerate(CHUNKS):
                for i in range(f0, f1):
                    field_chunk[i] = ci

            for t in range(NT):
                stage = {}
                remaining = {}
                for ci, (f0, f1) in enumerate(CHUNKS):
                    cols = sum(N_PAIRS[i] for i in range(f0, f1)) * D
                    stage[ci] = stage_pool.tile(
                        [128, cols], BF16, tag=f"st{ci}", name=f"st{t}_{ci}")
                    remaining[ci] = f1 - f0

                def mm_pieces(i):
                    """Emit matmul pieces for field i; returns psum tile."""
                    npair = N_PAIRS[i]
                    cols = npair * D
                    g = i % 2
                    r0 = g * D
                    k = i // 2
                    ci = field_chunk[i]
                    lhsT = xt_sb[t][r0:r0 + D, k * 128:(k + 1) * 128]
                    woff0 = WOFF[i] - WOFF[chunk_lo[ci][0] if g == 0
                                           else chunk_hi[ci][0]]
                    ps = psum_pool.tile([128, cols], F32, tag="ps",
                                        name=f"ps{t}_{i}")
                    pieces = []
                    for s0 in range(0, cols, 512):
                        n = min(512, cols - s0)
                        pieces.append((ps[:, s0:s0 + n], lhsT,
                                       w_sb[ci][r0:r0 + D,
                                                woff0 + s0:woff0 + s0 + n]))
                    return ps, pieces

                def consume(i, ps):
                    """Emit drain+mul (or direct mul) for field i."""
                    npair = N_PAIRS[i]
                    cols = npair * D
                    ci = field_chunk[i]
                    st = stage[ci]
                    c0 = (P_START[i] - P_START[CHUNKS[ci][0]]) * D
                    dst = st[:, c0:c0 + cols]
                    xj = xn_sb[t][:, (i + 1) * D:(i + 1 + npair) * D]
                    if DRAINED[i]:
                        nc.scalar.copy(out=dst, in_=ps[:])
                        nc.vector.tensor_mul(dst, dst, xj)
                    else:
                        nc.vector.tensor_mul(dst, ps[:], xj)
                    remaining[ci] -= 1
                    if remaining[ci] == 0:
                        f0, f1 = CHUNKS[ci]
                        cc0 = P_START[f0] * D
                        ccols = sum(N_PAIRS[j] for j in range(f0, f1)) * D
                        nc.sync.dma_start(
                            out=out[t * 128:(t + 1) * 128, cc0:cc0 + ccols],
                            in_=st[:])

                # Fields in pairs (2k, 2k+1): interleave matmul pieces so the
                # two PE row groups run concurrently, then emit consumers
                # (direct-mul fields first so DVE isn't blocked on a drain).
                for k in range(13):
                    fa, fb = 2 * k, 2 * k + 1
                    ps_a, pieces_a = mm_pieces(fa)
                    if fb < NF:
                        ps_b, pieces_b = mm_pieces(fb)
                    else:
                        ps_b, pieces_b = None, []
                    for pi in range(max(len(pieces_a), len(pieces_b))):
                        for pieces in (pieces_a, pieces_b):
                            if pi < len(pieces):
                                o, l, r = pieces[pi]
                                nc.tensor.matmul(o, l, r, start=True, stop=True)
                    order = [fa] if fb >= NF else (
                        [fa, fb] if DRAINED[fb] and not DRAINED[fa]
                        else [fb, fa])
                    for i in order:
                        consume(i, ps_a if i == fa else ps_b)
    nc.compile()
    return nc


def prep_inputs(x: np.ndarray, W: np.ndarray):
    """Full inputs -> per-core in_maps with pre-packed bf16 layouts."""
    x = np.ascontiguousarray(np.asarray(x, dtype=np.float32))
    W = np.ascontiguousarray(np.asarray(W, dtype=np.float32))
    # Pair-grouped weights: wg[:, p*64+e] = W[p][:, e]; split by parity of
    # the left field into the two PE row-group operand tensors.
    wg = W.transpose(1, 0, 2).reshape(D, OUT_COLS)
    wl = np.concatenate(
        [wg[:, P_START[i] * D:(P_START[i] + N_PAIRS[i]) * D] for i in EVEN],
        axis=1).astype(ml_dtypes.bfloat16)
    wh = np.concatenate(
        [wg[:, P_START[i] * D:(P_START[i] + N_PAIRS[i]) * D] for i in ODD],
        axis=1).astype(ml_dtypes.bfloat16)
    wl = np.ascontiguousarray(wl)
    wh = np.ascontiguousarray(wh)
    in_maps = []
    for c in range(NCORES):
        xc = x[:, c * BC:(c + 1) * BC, :]                      # [26, 512, 64]
        xn = np.ascontiguousarray(
            xc.transpose(1, 0, 2).reshape(BC, F * D).astype(ml_dtypes.bfloat16))
        xr = xc.reshape(F, NT, 128, D)
        xt_l = np.ascontiguousarray(
            xr[EVEN].transpose(3, 1, 0, 2)          # [64, NT, 13, 128]
            .reshape(D, NT * XT_LO_FIELDS * 128).astype(ml_dtypes.bfloat16))
        xt_h = np.ascontiguousarray(
            xr[ODD].transpose(3, 1, 0, 2)
            .reshape(D, NT * XT_HI_FIELDS * 128).astype(ml_dtypes.bfloat16))
        in_maps.append(
            {"xn": xn, "xt_lo": xt_l, "xt_hi": xt_h, "w_lo": wl, "w_hi": wh})
    return in_maps


_CACHED_NC = None


def kernel(x: np.ndarray, W: np.ndarray) -> np.ndarray:
    global _CACHED_NC
    from concourse.bass_utils import run_bass_kernel_spmd

    if _CACHED_NC is None:
        _CACHED_NC = build_bass()
    in_maps = prep_inputs(x, W)
    res = run_bass_kernel_spmd(_CACHED_NC, in_maps, list(range(NCORES)))
    shards = [
        np.asarray(res.results[c]["out"]).astype(np.float32) for c in range(NCORES)
    ]
    return np.concatenate(shards, axis=0)


# revision 5
# speedup vs baseline: 1.1645x; 1.1645x over previous
"""BiLinearInteraction Trainium2 kernel (8 NeuronCores, data-parallel over batch).

Reference computation (per pair p=(i,j) of F=26 fields, P=325 pairs):
    out[b, p*64:(p+1)*64] = (x[i, b, :] @ W[p]) * x[j, b, :]
Full shapes: x [26, 4096, 64] f32, W [325, 64, 64] f32 -> out [4096, 20800] f32.

Strategy (v3)
- Shard batch 4096 -> 8 x 512 (4 tiles of 128 rows/core), replicate W.
- HBM traffic ~28 MB/core: out 21.3MB bf16 write + reads ~6.6MB single-copy
  bf16. Even fields' matmul operands (lhsT xt, rhs w) sit in SBUF partitions
  0-63 and odd fields' in 64-127, so PE 2-row-group concurrency
  (tile_position row tiling) needs no duplicated HBM copies; consecutive
  fields' matmul pieces are emitted interleaved to pair the row groups.
- SWDGE descriptor generation costs ~850ns per dma_start serially on the Q7,
  so loads are consolidated to 9 issues: 7 per-chunk w loads (lo/hi packed in
  one [128, cmax] block, pad transferred on the narrow half) + tile-0
  (xn|xt) block + tiles-1-3 block. All input SBUF tiles are one-shot consts.
- Elementwise: measured rates ACT copy 0.833ns/el + 400ns/instr, DVE mul
  0.58ns/el from SBUF bf16 (2x mode, separate dst), 1.3ns/el from PSUM f32.
  Balanced split: the 12 biggest fields drain PSUM->bf16 cp tile on ACT then
  mul on DVE at 2x; the 13 smallest mul straight from PSUM. ~17.3/18.4us per
  tile on ACT/DVE.
- Output staged per (tile, chunk) and written as 7 contiguous bf16 DMAs/tile
  on the SP HWDGE ring; first chunk is one field so writes start early.
"""

import sys

sys.path.insert(0, "/opt/trn_rl_repo")

from itertools import combinations

import ml_dtypes
import numpy as np

import concourse.bass as bass
import concourse.mybir as mybir
from concourse import bacc
from concourse.tile import TileContext

F, D, B = 26, 64, 4096
NCORES = 8
BC = B // NCORES          # 512 batch rows per core
NT = BC // 128            # 4 batch tiles of 128 rows
NF = F - 1                # 25 left fields
PAIRS = list(combinations(range(F), 2))
N_PAIRS = [F - 1 - i for i in range(NF)]            # pairs with left field i
P_START = [sum(N_PAIRS[:i]) for i in range(NF)]     # first pair index of field i
P = sum(N_PAIRS)          # 325
OUT_COLS = P * D          # 20800

# column offset of field i inside the parity-packed w_lo / w_hi streams
WOFF = {}
_ol = _oh = 0
for _i in range(NF):
    if _i % 2 == 0:
        WOFF[_i] = _ol
        _ol += N_PAIRS[_i] * D
    else:
        WOFF[_i] = _oh
        _oh += N_PAIRS[_i] * D

# Output chunks: contiguous field ranges; first/last small for early writes
# and a short tail.
CHUNKS = [(0, 1), (1, 3), (3, 6), (6, 10), (10, 15), (15, 21), (21, 25)]
# per-chunk (first even field, lo cols), (first odd field, hi cols), padded max
CHUNK_LO, CHUNK_HI, CHUNK_MAX = [], [], []
for _f0, _f1 in CHUNKS:
    _ev = [i for i in range(_f0, _f1) if i % 2 == 0]
    _od = [i for i in range(_f0, _f1) if i % 2 == 1]
    _lc = sum(N_PAIRS[i] for i in _ev) * D
    _hc = sum(N_PAIRS[i] for i in _od) * D
    CHUNK_LO.append((_ev[0] if _ev else None, _lc))
    CHUNK_HI.append((_od[0] if _od else None, _hc))
    CHUNK_MAX.append(max(_lc, _hc))
W_PACK_COLS = sum(CHUNK_MAX)

N_DRAIN = 12              # fields 0..11 drained (ACT), 12..24 direct (DVE)
XT_BLK = (len([i for i in range(NF) if i % 2 == 0])) * 128   # 1664 lo cols
XX_TILE = F * D + XT_BLK  # 3328: [xn 1664 | xt 1664] per batch tile

F32 = mybir.dt.float32
BF16 = mybir.dt.bfloat16


def build_bass() -> bass.Bass:
    nc = bacc.Bacc()
    w = nc.declare_dram_parameter("w", [128, W_PACK_COLS], BF16, isOutput=False)
    xx0 = nc.declare_dram_parameter("xx0", [128, XX_TILE], BF16, isOutput=False)
    xx123 = nc.declare_dram_parameter(
        "xx123", [128, 3 * XX_TILE], BF16, isOutput=False)
    out = nc.declare_dram_parameter("out", [BC, OUT_COLS], BF16, isOutput=True)

    with TileContext(nc) as tc:
        with (
            tc.tile_pool(name="consts", bufs=1) as consts,
            tc.tile_pool(name="stage", bufs=2) as stage_pool,
            tc.tile_pool(name="cp_pool", bufs=3) as cp_pool,
            tc.tile_pool(name="psum", bufs=2, space="PSUM") as psum_pool,
        ):
            w_sb = [consts.tile([128, CHUNK_MAX[ci]], BF16,
                                tag=f"w{ci}", name=f"w{ci}")
                    for ci in range(len(CHUNKS))]
            xx0_sb = consts.tile([128, XX_TILE], BF16, tag="xx0", name="xx0")
            xx123_sb = consts.tile([128, 3 * XX_TILE], BF16,
                                   tag="xx123", name="xx123")

            # 9 SWDGE loads, just-in-time order: first chunk's weights and
            # tile-0 operands first; tiles 1-3 bulk last.
            _woff = [sum(CHUNK_MAX[:ci]) for ci in range(len(CHUNKS))]
            nc.gpsimd.dma_start(
                out=w_sb[0][:], in_=w[:, _woff[0]:_woff[0] + CHUNK_MAX[0]])
            nc.gpsimd.dma_start(out=xx0_sb[:], in_=xx0[:, :])
            for ci in range(1, len(CHUNKS)):
                nc.gpsimd.dma_start(
                    out=w_sb[ci][:], in_=w[:, _woff[ci]:_woff[ci] + CHUNK_MAX[ci]])
            nc.gpsimd.dma_start(out=xx123_sb[:], in_=xx123[:, :])

            field_chunk = {}
            for ci, (f0, f1) in enumerate(CHUNKS):
                for i in range(f0, f1):
                    field_chunk[i] = ci

            def xtile(t):
                return (xx0_sb, 0) if t == 0 else (xx123_sb, (t - 1) * XX_TILE)

            for t in range(NT):
                xsb, xbase = xtile(t)
                stage = {}
                remaining = {}
                for ci, (f0, f1) in enumerate(CHUNKS):
                    cols = sum(N_PAIRS[i] for i in range(f0, f1)) * D
                    stage[ci] = stage_pool.tile(
                        [128, cols], BF16, tag=f"st{ci}", name=f"st{t}_{ci}")
                    remaining[ci] = f1 - f0

                def mm_pieces(i):
                    npair = N_PAIRS[i]
                    cols = npair * D
                    g = i % 2
                    r0 = g * D
                    k = i // 2
                    ci = field_chunk[i]
                    lhsT = xsb[r0:r0 + D,
                               xbase + F * D + k * 128:xbase + F * D + (k + 1) * 128]
                    first = CHUNK_LO[ci][0] if g == 0 else CHUNK_HI[ci][0]
                    woff0 = WOFF[i] - WOFF[first]
                    ps = psum_pool.tile([128, cols], F32, tag="ps",
                                        name=f"ps{t}_{i}")
                    pieces = []
                    for s0 in range(0, cols, 512):
                        n = min(512, cols - s0)
                        pieces.append((ps[:, s0:s0 + n], lhsT,
                                       w_sb[ci][r0:r0 + D,
                                                woff0 + s0:woff0 + s0 + n]))
                    return ps, pieces

                def consume(i, ps):
                    npair = N_PAIRS[i]
                    cols = npair * D
                    ci = field_chunk[i]
                    st = stage[ci]
                    c0 = (P_START[i] - P_START[CHUNKS[ci][0]]) * D
                    dst = st[:, c0:c0 + cols]
                    xj = xsb[:, xbase + (i + 1) * D:xbase + (i + 1 + npair) * D]
                    if i < N_DRAIN:
                        cp = cp_pool.tile([128, cols], BF16, tag="cp",
                                          name=f"cp{t}_{i}")
                        nc.scalar.copy(out=cp[:], in_=ps[:])
                        nc.vector.tensor_mul(dst, cp[:], xj)
                    else:
                        nc.vector.tensor_mul(dst, ps[:], xj)
                    remaining[ci] -= 1
                    if remaining[ci] == 0:
                        f0, f1 = CHUNKS[ci]
                        cc0 = P_START[f0] * D
                        ccols = sum(N_PAIRS[j] for j in range(f0, f1)) * D
                        nc.sync.dma_start(
                            out=out[t * 128:(t + 1) * 128, cc0:cc0 + ccols],
                            in_=st[:])

                for k in range(13):
                    fa, fb = 2 * k, 2 * k + 1
                    ps_a, pieces_a = mm_pieces(fa)
                    if fb < NF:
                        ps_b, pieces_b = mm_pieces(fb)
                    else:
                        ps_b, pieces_b = None, []
                    for pi in range(max(len(pieces_a), len(pieces_b))):
                        for pieces in (pieces_a, pieces_b):
                            if pi < len(pieces):
                                o, l, r = pieces[pi]
                                nc.tensor.matmul(o, l, r, start=True, stop=True)
                    if fb >= NF:
                        order = [fa]
                    elif fb >= N_DRAIN and fa < N_DRAIN:
                        order = [fb, fa]   # direct field first on DVE
                    else:
                        order = [fa, fb]
                    for i in order:
                        consume(i, ps_a if i == fa else ps_b)
    nc.compile()
    return nc


def prep_inputs(x: np.ndarray, W: np.ndarray):
    """Full inputs -> per-core in_maps with pre-packed bf16 layouts."""
    x = np.ascontiguousarray(np.asarray(x, dtype=np.float32))
    W = np.ascontiguousarray(np.asarray(W, dtype=np.float32))
    # Pair-grouped weights wg[:, p*64+e] = W[p][:, e]; pack per chunk:
    # partitions 0-63 = even (lo) piece, 64-127 = odd (hi) piece, each
    # zero-padded to the chunk's max width.
    wg = W.transpose(1, 0, 2).reshape(D, OUT_COLS)
    wp = np.zeros((128, W_PACK_COLS), dtype=np.float32)
    col = 0
    for ci, (f0, f1) in enumerate(CHUNKS):
        lo = np.concatenate(
            [wg[:, P_START[i] * D:(P_START[i] + N_PAIRS[i]) * D]
             for i in range(f0, f1) if i % 2 == 0], axis=1)
        hi_parts = [wg[:, P_START[i] * D:(P_START[i] + N_PAIRS[i]) * D]
                    for i in range(f0, f1) if i % 2 == 1]
        wp[0:D, col:col + lo.shape[1]] = lo
        if hi_parts:
            hi = np.concatenate(hi_parts, axis=1)
            wp[D:2 * D, col:col + hi.shape[1]] = hi
        col += CHUNK_MAX[ci]
    wp = np.ascontiguousarray(wp.astype(ml_dtypes.bfloat16))

    EV = [i for i in range(NF) if i % 2 == 0]
    OD = [i for i in range(NF) if i % 2 == 1]
    in_maps = []
    for c in range(NCORES):
        xc = x[:, c * BC:(c + 1) * BC, :]                      # [26, 512, 64]
        xr = xc.reshape(F, NT, 128, D)
        xx = np.zeros((NT, 128, XX_TILE), dtype=np.float32)
        for t in range(NT):
            # xn block: [128, 26*64] batch-major field concat
            xx[t, :, :F * D] = xr[:, t].transpose(1, 0, 2).reshape(128, F * D)
            # xt block: [64, 13*128] per parity half (d-major lhsT layout)
            xtl = xr[EV, t].transpose(2, 0, 1).reshape(D, len(EV) * 128)
            xth = xr[OD, t].transpose(2, 0, 1).reshape(D, len(OD) * 128)
            xx[t, 0:D, F * D:F * D + xtl.shape[1]] = xtl
            xx[t, D:2 * D, F * D:F * D + xth.shape[1]] = xth
        xxb = xx.astype(ml_dtypes.bfloat16)
        in_maps.append({
            "w": wp,
            "xx0": np.ascontiguousarray(xxb[0]),
            "xx123": np.ascontiguousarray(
                xxb[1:].transpose(1, 0, 2).reshape(128, 3 * XX_TILE)),
        })
    return in_maps


_CACHED_NC = None


def kernel(x: np.ndarray, W: np.ndarray) -> np.ndarray:
    global _CACHED_NC
    from concourse.bass_utils import run_bass_kernel_spmd

    if _CACHED_NC is None:
        _CACHED_NC = build_bass()
    in_maps = prep_inputs(x, W)
    res = run_bass_kernel_spmd(_CACHED_NC, in_maps, list(range(NCORES)))
    shards = [
        np.asarray(res.results[c]["out"]).astype(np.float32) for c in range(NCORES)
    ]
    return np.concatenate(shards, axis=0)
